# revision 37
# baseline (speedup 1.0000x reference)
"""CRF negative-log-likelihood kernel for Trainium2 (8 NeuronCores, batch-sharded).

Algorithm:
  - Launch 1 (vocab-sharded): t2 = embedding @ fc_w in fp8 (x512 input
    scaling) from a host-pretransposed embedding, fp32 out in block-major
    layout (contiguous output DMAs).
  - Launch 2 (batch-sharded, 8 rows/core): 6 big indirect-DMA gathers fetch
    16 bf16 per token from t2 (one 32B descriptor per token); bf16 PE
    block-transposes to class-on-partition; exp on ACT with per-partition
    log-colsum bias folded in; segmented linear-space forward/backward scan
    with L=2 steps x S=2048 segments in bf16 (one matmul+multiply per
    direction per slice).  Tokens are host-permuted k-major so scan slices
    are contiguous; pieces are ordered and scan ops emitted at readiness
    points so the tail after the last gather is ~2 multiplies.
  - Host (float64, vectorized): gold-path numerator from t2, rank-1 junction
    chain across segments (d divided by colsum to undo the bias fold),
    exact ragged-tail recompute, final assembly.
"""
import sys
sys.path.insert(0, "/opt/trn_rl_repo")
import numpy as np
import ml_dtypes
from contextlib import ExitStack

import concourse.bass as bass
import concourse.bacc as bacc_mod
import concourse.mybir as mybir
import concourse.tile as tile
from concourse.masks import make_identity
from concourse.bass_utils import run_bass_kernel_spmd

F32 = mybir.dt.float32
BF16 = mybir.dt.bfloat16
FP8 = mybir.dt.float8e4
I32 = mybir.dt.int32
BF = ml_dtypes.bfloat16
F8 = ml_dtypes.float8_e4m3

V, E, C = 50257, 128, 16
B, T = 64, 4096
L, S = 2, 2048
VPAD = 51200
VSH = VPAD // 8
BL = 8
NCORES = 8
H = 2                       # independent scan chains (halves of S)
SH = S // H

LAST_EXEC_NS = {}
_TRACE = False
_CACHE = {}
LAST_RESULTS = {}


def build_t2_kernel():
    nc = bacc_mod.Bacc()
    embT = nc.dram_tensor("embT", [E, VSH], FP8, kind="ExternalInput")
    fc_w = nc.dram_tensor("fc_w", [E, C], FP8, kind="ExternalInput")
    # block-major layout [p, blk, c]; host reorders to (VSH, C)
    t2_s = nc.dram_tensor("t2_s", [128, VSH // 128 * C], F32,
                          kind="ExternalOutput")

    nblk = VSH // 128        # 50 blocks of 128 vocab rows
    CHUNKS = [4, 10, 12, 12, 12]   # blocks per input chunk (small first)
    with ExitStack() as ctx:
        tc = ctx.enter_context(tile.TileContext(nc))
        singles = ctx.enter_context(tc.tile_pool(name="singles", bufs=1))
        psum = ctx.enter_context(tc.tile_pool(name="psum", bufs=2, space="PSUM"))

        fcw_sb = singles.tile([E, C], FP8)
        nc.scalar.dma_start(out=fcw_sb[:], in_=fc_w[:])
        EMB = singles.tile([E, VSH], FP8)
        T2 = singles.tile([128, nblk * C], F32)
        b0 = 0
        for ch, nb in enumerate(CHUNKS):
            c0 = b0 * 128
            eng = nc.sync if ch % 2 == 0 else nc.scalar
            eng.dma_start(out=EMB[:, c0:c0 + nb * 128],
                          in_=embT[:, c0:c0 + nb * 128])
            b0 += nb
        b0 = 0
        for ch, nb in enumerate(CHUNKS):
            ps = psum.tile([128, 12 * C], F32, tag="ps")
            for i in range(nb):
                blk = b0 + i
                nc.tensor.matmul(ps[:, i * C:(i + 1) * C],
                                 lhsT=EMB[:, blk * 128:(blk + 1) * 128],
                                 rhs=fcw_sb[:], start=True, stop=True)
            c0 = b0 * C
            nc.vector.tensor_copy(T2[:, c0:c0 + nb * C], ps[:, 0:nb * C])
            eng = nc.sync if ch % 2 == 0 else nc.scalar
            eng.dma_start(out=t2_s[:, c0:c0 + nb * C], in_=T2[:, c0:c0 + nb * C])
            b0 += nb
    return nc


def build_main_kernel():
    nc = bacc_mod.Bacc()
    x_t = nc.dram_tensor("x_t", [128, T // 128 * BL], I32, kind="ExternalInput")
    t2 = nc.dram_tensor("t2", [VPAD, C], BF16, kind="ExternalInput")
    blockP = nc.dram_tensor("blockP", [128, 128], BF16, kind="ExternalInput")
    blockPT = nc.dram_tensor("blockPT", [128, 128], BF16, kind="ExternalInput")
    logcolsum = nc.dram_tensor("logcolsum", [128, 1], F32, kind="ExternalInput")
    sadj = nc.dram_tensor("sadj", [128, 1], BF16, kind="ExternalInput")

    r_out = nc.dram_tensor("r_out", [128, S], BF16, kind="ExternalOutput")
    d_out = nc.dram_tensor("d_out", [128, S], BF16, kind="ExternalOutput")

    with ExitStack() as ctx:
        tc = ctx.enter_context(tile.TileContext(nc))
        singles = ctx.enter_context(tc.tile_pool(name="singles", bufs=1))
        big = ctx.enter_context(tc.tile_pool(name="big", bufs=1))
        psumT = ctx.enter_context(tc.tile_pool(name="psumT", bufs=2, space="PSUM"))
        psumS = ctx.enter_context(tc.tile_pool(name="psumS", bufs=2, space="PSUM"))

        xt_sb = singles.tile([128, T // 128 * BL], I32)
        blockP_sb = singles.tile([128, 128], BF16)
        nc.scalar.dma_start(out=blockP_sb[:], in_=blockP[:])
        blockPT_sb = singles.tile([128, 128], BF16)
        nc.scalar.dma_start(out=blockPT_sb[:], in_=blockPT[:])
        logcolsum_sb = singles.tile([128, 1], F32)
        nc.scalar.dma_start(out=logcolsum_sb[:], in_=logcolsum[:])
        sadj_sb = singles.tile([128, 1], BF16)
        nc.scalar.dma_start(out=sadj_sb[:], in_=sadj[:])

        TM = big.tile([128, T], BF16)
        EXPG = big.tile([128, T], BF16)
        ident = singles.tile([128, 128], BF16)

        # --- gather (5 asymmetric indirect DMAs) + PE transpose + exp ---
        # k=0 region gets exp(x + log colsum) = colsum*w0 (host divides d by
        # colsum to compensate).  Pieces ordered so the scan's h1 chains run
        # while the h0 pieces are still in flight; scan ops are emitted at
        # their readiness points (engines execute in order).
        r_sb = big.tile([128, S], BF16)
        d_sb = big.tile([128, S], BF16)

        def gather_dma(c0, w, xt_eng=None):
            (xt_eng or nc.sync).dma_start(
                out=xt_sb[:, c0 // 16:(c0 + w) // 16],
                in_=x_t[:, c0 // 16:(c0 + w) // 16])
            nc.gpsimd.indirect_dma_start(
                out=TM[:, c0:c0 + w],
                out_offset=None,
                in_=t2[:],
                in_offset=bass.IndirectOffsetOnAxis(
                    ap=xt_sb[:, c0 // 16:(c0 + w) // 16], axis=0),
            )

        def gather_piece(c0, w, dma=True):
            if dma:
                gather_dma(c0, w)
            bias = logcolsum_sb[:] if c0 < S else 0.0
            for base in range(c0, c0 + w, 512):
                gw = min(512, c0 + w - base)
                psT = psumT.tile([128, 512], BF16, tag="psT")
                for q in range(gw // 128):
                    th = base // 128 + q
                    nc.tensor.transpose(psT[:, q * 128:(q + 1) * 128],
                                        TM[:, th * 128:(th + 1) * 128], ident[:])
                nc.scalar.activation(EXPG[:, base:base + gw], psT[:, 0:gw],
                                     mybir.ActivationFunctionType.Exp, bias=bias)

        # scan pieces, L=2:  r = (blkP^T @ colsum*w0) * w1
        #                    d = colsum*w0 * (blkPT^T @ w1)
        def mmR(a, w=512):
            ps = psumS.tile([128, 512], F32, tag="psR")
            nc.tensor.matmul(ps[:, 0:w], lhsT=blockP_sb[:], rhs=EXPG[:, a:a + w],
                             start=True, stop=True)
            return ps

        def mmD(a, w=512):
            ps = psumS.tile([128, 512], F32, tag="psD")
            nc.tensor.matmul(ps[:, 0:w], lhsT=blockPT_sb[:],
                             rhs=EXPG[:, S + a:S + a + w],
                             start=True, stop=True)
            return ps

        def mulR(a, ps, w=512):
            nc.vector.tensor_mul(r_sb[:, a:a + w], ps[:, 0:w],
                                 EXPG[:, S + a:S + a + w])

        def mulD(a, ps, w=512, off=0):
            nc.vector.tensor_mul(d_sb[:, a:a + w], ps[:, off:off + w],
                                 EXPG[:, a:a + w])

        gather_dma(1024, 1024, xt_eng=nc.gpsimd)   # first offsets via SWDGE:
        make_identity(nc, ident[:])    # same-queue completion, shorter wait
        gather_piece(1024, 1024, dma=False)
        gather_piece(3072, 1024)       # k1 h1
        # h1 chains: inputs complete
        psR1a = mmR(1024); psR1b = mmR(1536)
        psD1a = mmD(1024); psD1b = mmD(1536)
        mulR(1024, psR1a); mulR(1536, psR1b)
        mulD(1024, psD1a); mulD(1536, psD1b)
        nc.sync.dma_start(out=r_out[:, SH:S], in_=r_sb[:, SH:S])
        nc.scalar.dma_start(out=d_out[:, SH:S], in_=d_sb[:, SH:S])

        gather_piece(0, 512)           # k0 h0 first half (contains col 0)
        nc.vector.tensor_mul(EXPG[:, 0:1], EXPG[:, 0:1], sadj_sb[:])
        gather_piece(2048, 1024)       # k1 h0
        psD0a = mmD(0); psD0b = mmD(512)
        psR0a = mmR(0)                 # rhs = cols 0..511 (piece 1 + sadj)
        mulR(0, psR0a)
        mulD(0, psD0a)
        nc.sync.dma_start(out=r_out[:, 0:512], in_=r_sb[:, 0:512])
        nc.scalar.dma_start(out=d_out[:, 0:512], in_=d_sb[:, 0:512])

        gather_piece(512, 256)         # k0 h0 third quarter
        psR0b = mmR(512, 256)
        mulR(512, psR0b, 256)
        mulD(512, psD0b, 256)
        nc.sync.dma_start(out=r_out[:, 512:768], in_=r_sb[:, 512:768])
        nc.scalar.dma_start(out=d_out[:, 512:768], in_=d_sb[:, 512:768])

        gather_piece(768, 256)         # k0 h0 last quarter
        psR0c = mmR(768, 256)
        mulR(768, psR0c, 256)
        mulD(768, psD0b, 256, off=256)
        nc.sync.dma_start(out=r_out[:, 768:SH], in_=r_sb[:, 768:SH])
        nc.scalar.dma_start(out=d_out[:, 768:SH], in_=d_sb[:, 768:SH])
    return nc


def _host_prep(embedding, fc_w, fc_b, trans, start):
    emb_pad = np.zeros((VPAD, E), np.float32)
    emb_pad[:V] = embedding
    embT_pad = np.ascontiguousarray(emb_pad.T * 64.0).astype(F8)
    P_eff64 = np.exp(trans.astype(np.float64) + fc_b[None, :].astype(np.float64))
    colsum64 = P_eff64.sum(0)
    start_adj = np.exp(start.astype(np.float64) + fc_b) / colsum64
    P_eff32 = P_eff64.astype(np.float32)

    eye8 = np.eye(BL, dtype=np.float32)
    return dict(
        embT_pad=embT_pad,
        P_eff=P_eff64,
        blockP=np.ascontiguousarray(np.kron(eye8, P_eff32)).astype(BF),
        blockPT=np.ascontiguousarray(np.kron(eye8, P_eff32.T.copy())).astype(BF),
        logcolsum=np.log(np.tile(colsum64, BL))[:, None].astype(np.float32),
        log_sadj=np.log(start_adj),
        sadj=np.tile(start_adj, BL)[:, None].astype(BF),
    )


def _run(nc, in_maps, label):
    res = run_bass_kernel_spmd(nc, in_maps, core_ids=list(range(NCORES)),
                               trace=_TRACE)
    if res.exec_time_ns is not None:
        LAST_EXEC_NS[label] = res.exec_time_ns
    LAST_RESULTS[label] = res
    return res.results


def kernel(x, tags, embedding, fc_w, fc_b, start_transitions, end_transitions,
           transitions):
    x = np.asarray(x, np.int32)
    tags = np.asarray(tags, np.int32)
    embedding = np.asarray(embedding, np.float32)
    fc_w = np.asarray(fc_w, np.float32)
    fc_b = np.asarray(fc_b, np.float32)
    trans = np.asarray(transitions, np.float32)
    start = np.asarray(start_transitions, np.float32)
    end = np.asarray(end_transitions, np.float32)

    prep = _host_prep(embedding, fc_w, fc_b, trans, start)

    if "t2" not in _CACHE:
        nc1 = build_t2_kernel()
        nc1.finalize()
        _CACHE["t2"] = nc1
    if "main" not in _CACHE:
        nc2 = build_main_kernel()
        nc2.finalize()
        _CACHE["main"] = nc2

    # ---- launch 1: t2 = emb_pad @ fc_w, vocab-sharded (fp8, x512 scaled) ----
    fcw_f8 = (fc_w * 8.0).astype(F8)
    in1 = [{"embT": np.ascontiguousarray(prep["embT_pad"][:, k * VSH:(k + 1) * VSH]),
            "fc_w": fcw_f8} for k in range(NCORES)]
    res1 = _run(_CACHE["t2"], in1, "t2")
    t2_full = np.concatenate(
        [np.asarray(res1[k]["t2_s"]).reshape(128, VSH // 128, C)
         .transpose(1, 0, 2).reshape(VSH, C) for k in range(NCORES)], axis=0)
    t2_full = np.ascontiguousarray(t2_full, dtype=np.float32) * (1.0 / 512.0)

    # ---- launch 2: main kernel, batch-sharded ----
    t2_bf = t2_full.astype(BF)
    # token permutation: G column (th*128+p) holds token t = (col%S)*L + col//S
    cols = np.arange(T)
    tperm = (cols % S) * L + cols // S
    xp = x[:, tperm].copy()
    in2 = []
    for k in range(NCORES):
        sl = slice(k * BL, (k + 1) * BL)
        xt = xp[sl].reshape(BL, T // 128, 128).transpose(2, 1, 0) \
                   .reshape(128, T // 128 * BL)
        in2.append({
            "x_t": np.ascontiguousarray(xt),
            "t2": t2_bf,
            "blockP": prep["blockP"], "blockPT": prep["blockPT"],
            "logcolsum": prep["logcolsum"], "sadj": prep["sadj"],
        })
    res2 = _run(_CACHE["main"], in2, "main")

    # ---- host combine (float64) ----
    lengths = (x != 0).sum(1)
    start64 = start.astype(np.float64)
    end64 = end.astype(np.float64)
    fcb64 = fc_b.astype(np.float64)
    trans64 = trans.astype(np.float64)
    Pe = prep["P_eff"]
    colsum64 = Pe.sum(0)
    t264 = t2_full.astype(np.float64)
    exp_end = np.exp(end64)

    # numerator: gold-path score, fully vectorized on host
    maskf = (x != 0).astype(np.float64)
    em_tag = t264[x, tags] + fcb64[tags]           # (B,T)
    num = start64[tags[:, 0]] + (em_tag * maskf).sum(1)
    num += (trans64[tags[:, :-1], tags[:, 1:]] * maskf[:, 1:]).sum(1)
    last_tags = tags[np.arange(B), lengths - 1]
    num += end64[last_tags]

    total = 0.0
    for core in range(NCORES):
        r = np.asarray(res2[core]["r_out"], np.float64).reshape(BL, C, S)
        d = np.asarray(res2[core]["d_out"], np.float64).reshape(BL, C, S)
        d = d / colsum64[None, :, None]
        c = np.einsum('ij,bjs->bis', Pe, d)
        A = np.einsum('bis,bis->bs', r[:, :, :-1], c[:, :, 1:])   # junction s=1..S-1
        Bs = r.sum(axis=1)                                        # (BL, S)
        J = np.log(A) - np.log(Bs[:, 1:])                         # J[:, s-1] <-> junction s
        Jcum = np.concatenate([np.zeros((BL, 1)), np.cumsum(J, axis=1)], axis=1)
        for b in range(BL):
            gb = core * BL + b
            ln = int(lengths[gb])
            sstar = (ln - 1) // L
            logZ = Jcum[b, sstar - 1]        # junctions s=1..sstar-1
            alpha = r[b, :, sstar - 1].copy()
            for t in range(sstar * L, ln):
                w = np.exp(t264[x[gb, t]] + fcb64)
                alpha = (alpha @ Pe) * w
            logZ += np.log(alpha @ exp_end)
            total += -(num[gb] - logZ)
    return np.array(total, dtype=np.float32)


# revision 38
# speedup vs baseline: 1.0240x; 1.0240x over previous
"""CRF negative-log-likelihood kernel for Trainium2 (8 NeuronCores, batch-sharded).

Algorithm:
  - Launch 1 (vocab-sharded): t2 = embedding @ fc_w in fp8 (x512 input
    scaling) from a host-pretransposed embedding, fp32 out in block-major
    layout (contiguous output DMAs).
  - Launch 2 (batch-sharded, 8 rows/core): 6 big indirect-DMA gathers fetch
    16 bf16 per token from t2 (one 32B descriptor per token); bf16 PE
    block-transposes to class-on-partition; exp on ACT with per-partition
    log-colsum bias folded in; segmented linear-space forward/backward scan
    with L=2 steps x S=2048 segments in bf16 (one matmul+multiply per
    direction per slice).  Tokens are host-permuted k-major so scan slices
    are contiguous; pieces are ordered and scan ops emitted at readiness
    points so the tail after the last gather is ~2 multiplies.
  - Host (float64, vectorized): gold-path numerator from t2, rank-1 junction
    chain across segments (d divided by colsum to undo the bias fold),
    exact ragged-tail recompute, final assembly.
"""
import sys
sys.path.insert(0, "/opt/trn_rl_repo")
import numpy as np
import ml_dtypes
from contextlib import ExitStack

import concourse.bass as bass
import concourse.bacc as bacc_mod
import concourse.mybir as mybir
import concourse.tile as tile
from concourse.masks import make_identity
from concourse.bass_utils import run_bass_kernel_spmd

F32 = mybir.dt.float32
BF16 = mybir.dt.bfloat16
FP8 = mybir.dt.float8e4
I32 = mybir.dt.int32
BF = ml_dtypes.bfloat16
F8 = ml_dtypes.float8_e4m3

V, E, C = 50257, 128, 16
B, T = 64, 4096
L, S = 2, 2048
VPAD = 51200
VSH = VPAD // 8
BL = 8
NCORES = 8
H = 2                       # independent scan chains (halves of S)
SH = S // H

LAST_EXEC_NS = {}
_TRACE = False
_CACHE = {}
LAST_RESULTS = {}


def build_t2_kernel():
    nc = bacc_mod.Bacc()
    embT = nc.dram_tensor("embT", [E, VSH], FP8, kind="ExternalInput")
    fc_w = nc.dram_tensor("fc_w", [E, C], FP8, kind="ExternalInput")
    # block-major layout [p, blk, c]; host reorders to (VSH, C)
    t2_s = nc.dram_tensor("t2_s", [128, VSH // 128 * C], F32,
                          kind="ExternalOutput")

    nblk = VSH // 128        # 50 blocks of 128 vocab rows
    CHUNKS = [4, 10, 12, 12, 12]   # blocks per input chunk (small first)
    with ExitStack() as ctx:
        tc = ctx.enter_context(tile.TileContext(nc))
        singles = ctx.enter_context(tc.tile_pool(name="singles", bufs=1))
        psum = ctx.enter_context(tc.tile_pool(name="psum", bufs=2, space="PSUM"))

        fcw_sb = singles.tile([E, C], FP8)
        nc.scalar.dma_start(out=fcw_sb[:], in_=fc_w[:])
        EMB = singles.tile([E, VSH], FP8)
        T2 = singles.tile([128, nblk * C], F32)
        b0 = 0
        for ch, nb in enumerate(CHUNKS):
            c0 = b0 * 128
            eng = nc.sync if ch % 2 == 0 else nc.scalar
            eng.dma_start(out=EMB[:, c0:c0 + nb * 128],
                          in_=embT[:, c0:c0 + nb * 128])
            b0 += nb
        b0 = 0
        for ch, nb in enumerate(CHUNKS):
            ps = psum.tile([128, 12 * C], F32, tag="ps")
            for i in range(nb):
                blk = b0 + i
                nc.tensor.matmul(ps[:, i * C:(i + 1) * C],
                                 lhsT=EMB[:, blk * 128:(blk + 1) * 128],
                                 rhs=fcw_sb[:], start=True, stop=True)
            c0 = b0 * C
            nc.vector.tensor_copy(T2[:, c0:c0 + nb * C], ps[:, 0:nb * C])
            eng = nc.sync if ch % 2 == 0 else nc.scalar
            eng.dma_start(out=t2_s[:, c0:c0 + nb * C], in_=T2[:, c0:c0 + nb * C])
            b0 += nb
    return nc


def build_main_kernel():
    nc = bacc_mod.Bacc()
    x_t = nc.dram_tensor("x_t", [128, T // 128 * BL], I32, kind="ExternalInput")
    t2 = nc.dram_tensor("t2", [VPAD, C], BF16, kind="ExternalInput")
    blockP = nc.dram_tensor("blockP", [128, 128], BF16, kind="ExternalInput")
    blockPT = nc.dram_tensor("blockPT", [128, 128], BF16, kind="ExternalInput")
    logcolsum = nc.dram_tensor("logcolsum", [128, 1], F32, kind="ExternalInput")
    sadj = nc.dram_tensor("sadj", [128, 1], BF16, kind="ExternalInput")

    r_out = nc.dram_tensor("r_out", [128, S], BF16, kind="ExternalOutput")
    d_out = nc.dram_tensor("d_out", [128, S], BF16, kind="ExternalOutput")

    with ExitStack() as ctx:
        tc = ctx.enter_context(tile.TileContext(nc))
        singles = ctx.enter_context(tc.tile_pool(name="singles", bufs=1))
        big = ctx.enter_context(tc.tile_pool(name="big", bufs=1))
        psumT = ctx.enter_context(tc.tile_pool(name="psumT", bufs=2, space="PSUM"))
        psumS = ctx.enter_context(tc.tile_pool(name="psumS", bufs=2, space="PSUM"))

        xt_sb = singles.tile([128, T // 128 * BL], I32)
        blockP_sb = singles.tile([128, 128], BF16)
        nc.scalar.dma_start(out=blockP_sb[:], in_=blockP[:])
        blockPT_sb = singles.tile([128, 128], BF16)
        nc.scalar.dma_start(out=blockPT_sb[:], in_=blockPT[:])
        logcolsum_sb = singles.tile([128, 1], F32)
        nc.scalar.dma_start(out=logcolsum_sb[:], in_=logcolsum[:])
        sadj_sb = singles.tile([128, 1], BF16)
        nc.scalar.dma_start(out=sadj_sb[:], in_=sadj[:])

        TM = big.tile([128, T], BF16)
        EXPG = big.tile([128, T], BF16)
        ident = singles.tile([128, 128], BF16)

        # --- gather (5 asymmetric indirect DMAs) + PE transpose + exp ---
        # k=0 region gets exp(x + log colsum) = colsum*w0 (host divides d by
        # colsum to compensate).  Pieces ordered so the scan's h1 chains run
        # while the h0 pieces are still in flight; scan ops are emitted at
        # their readiness points (engines execute in order).
        r_sb = big.tile([128, S], BF16)
        d_sb = big.tile([128, S], BF16)

        def gather_dma(c0, w):
            nc.sync.dma_start(out=xt_sb[:, c0 // 16:(c0 + w) // 16],
                              in_=x_t[:, c0 // 16:(c0 + w) // 16])
            nc.gpsimd.indirect_dma_start(
                out=TM[:, c0:c0 + w],
                out_offset=None,
                in_=t2[:],
                in_offset=bass.IndirectOffsetOnAxis(
                    ap=xt_sb[:, c0 // 16:(c0 + w) // 16], axis=0),
            )

        def gather_piece(c0, w, dma=True):
            if dma:
                gather_dma(c0, w)
            bias = logcolsum_sb[:] if c0 < S else 0.0
            for base in range(c0, c0 + w, 512):
                gw = min(512, c0 + w - base)
                psT = psumT.tile([128, 512], BF16, tag="psT")
                for q in range(gw // 128):
                    th = base // 128 + q
                    nc.tensor.transpose(psT[:, q * 128:(q + 1) * 128],
                                        TM[:, th * 128:(th + 1) * 128], ident[:])
                nc.scalar.activation(EXPG[:, base:base + gw], psT[:, 0:gw],
                                     mybir.ActivationFunctionType.Exp, bias=bias)

        # scan pieces, L=2:  r = (blkP^T @ colsum*w0) * w1
        #                    d = colsum*w0 * (blkPT^T @ w1)
        def mmR(a, w=512):
            ps = psumS.tile([128, 512], F32, tag="psR")
            nc.tensor.matmul(ps[:, 0:w], lhsT=blockP_sb[:], rhs=EXPG[:, a:a + w],
                             start=True, stop=True)
            return ps

        def mmD(a, w=512):
            ps = psumS.tile([128, 512], F32, tag="psD")
            nc.tensor.matmul(ps[:, 0:w], lhsT=blockPT_sb[:],
                             rhs=EXPG[:, S + a:S + a + w],
                             start=True, stop=True)
            return ps

        def mulR(a, ps, w=512):
            nc.vector.tensor_mul(r_sb[:, a:a + w], ps[:, 0:w],
                                 EXPG[:, S + a:S + a + w])

        def mulD(a, ps, w=512, off=0):
            nc.vector.tensor_mul(d_sb[:, a:a + w], ps[:, off:off + w],
                                 EXPG[:, a:a + w])

        gather_dma(1024, 1024)         # k0 h1: issue first, before the
        make_identity(nc, ident[:])    # identity iota occupies GpSimd
        gather_piece(1024, 1024, dma=False)
        gather_piece(3072, 1024)       # k1 h1
        # h1 chains: inputs complete
        psR1a = mmR(1024); psR1b = mmR(1536)
        psD1a = mmD(1024); psD1b = mmD(1536)
        mulR(1024, psR1a); mulR(1536, psR1b)
        mulD(1024, psD1a); mulD(1536, psD1b)
        nc.sync.dma_start(out=r_out[:, SH:S], in_=r_sb[:, SH:S])
        nc.scalar.dma_start(out=d_out[:, SH:S], in_=d_sb[:, SH:S])

        gather_piece(0, 512)           # k0 h0 first half (contains col 0)
        nc.vector.tensor_mul(EXPG[:, 0:1], EXPG[:, 0:1], sadj_sb[:])
        gather_piece(2048, 1024)       # k1 h0
        psD0a = mmD(0); psD0b = mmD(512)
        psR0a = mmR(0)                 # rhs = cols 0..511 (piece 1 + sadj)
        mulR(0, psR0a)
        mulD(0, psD0a)
        nc.sync.dma_start(out=r_out[:, 0:512], in_=r_sb[:, 0:512])
        nc.scalar.dma_start(out=d_out[:, 0:512], in_=d_sb[:, 0:512])

        gather_piece(512, 256)         # k0 h0 third quarter
        psR0b = mmR(512, 256)
        mulR(512, psR0b, 256)
        mulD(512, psD0b, 256)
        nc.sync.dma_start(out=r_out[:, 512:768], in_=r_sb[:, 512:768])
        nc.scalar.dma_start(out=d_out[:, 512:768], in_=d_sb[:, 512:768])

        gather_piece(768, 256)         # k0 h0 last quarter
        psR0c = mmR(768, 256)
        mulR(768, psR0c, 256)
        mulD(768, psD0b, 256, off=256)
        nc.sync.dma_start(out=r_out[:, 768:SH], in_=r_sb[:, 768:SH])
        nc.scalar.dma_start(out=d_out[:, 768:SH], in_=d_sb[:, 768:SH])
    return nc


def _host_prep(embedding, fc_w, fc_b, trans, start):
    emb_pad = np.zeros((VPAD, E), np.float32)
    emb_pad[:V] = embedding
    embT_pad = np.ascontiguousarray(emb_pad.T * 64.0).astype(F8)
    P_eff64 = np.exp(trans.astype(np.float64) + fc_b[None, :].astype(np.float64))
    colsum64 = P_eff64.sum(0)
    start_adj = np.exp(start.astype(np.float64) + fc_b) / colsum64
    P_eff32 = P_eff64.astype(np.float32)

    eye8 = np.eye(BL, dtype=np.float32)
    return dict(
        embT_pad=embT_pad,
        P_eff=P_eff64,
        blockP=np.ascontiguousarray(np.kron(eye8, P_eff32)).astype(BF),
        blockPT=np.ascontiguousarray(np.kron(eye8, P_eff32.T.copy())).astype(BF),
        logcolsum=np.log(np.tile(colsum64, BL))[:, None].astype(np.float32),
        log_sadj=np.log(start_adj),
        sadj=np.tile(start_adj, BL)[:, None].astype(BF),
    )


def _run(nc, in_maps, label):
    res = run_bass_kernel_spmd(nc, in_maps, core_ids=list(range(NCORES)),
                               trace=_TRACE)
    if res.exec_time_ns is not None:
        LAST_EXEC_NS[label] = res.exec_time_ns
    LAST_RESULTS[label] = res
    return res.results


def kernel(x, tags, embedding, fc_w, fc_b, start_transitions, end_transitions,
           transitions):
    x = np.asarray(x, np.int32)
    tags = np.asarray(tags, np.int32)
    embedding = np.asarray(embedding, np.float32)
    fc_w = np.asarray(fc_w, np.float32)
    fc_b = np.asarray(fc_b, np.float32)
    trans = np.asarray(transitions, np.float32)
    start = np.asarray(start_transitions, np.float32)
    end = np.asarray(end_transitions, np.float32)

    prep = _host_prep(embedding, fc_w, fc_b, trans, start)

    if "t2" not in _CACHE:
        nc1 = build_t2_kernel()
        nc1.finalize()
        _CACHE["t2"] = nc1
    if "main" not in _CACHE:
        nc2 = build_main_kernel()
        nc2.finalize()
        _CACHE["main"] = nc2

    # ---- launch 1: t2 = emb_pad @ fc_w, vocab-sharded (fp8, x512 scaled) ----
    fcw_f8 = (fc_w * 8.0).astype(F8)
    in1 = [{"embT": np.ascontiguousarray(prep["embT_pad"][:, k * VSH:(k + 1) * VSH]),
            "fc_w": fcw_f8} for k in range(NCORES)]
    res1 = _run(_CACHE["t2"], in1, "t2")
    t2_full = np.concatenate(
        [np.asarray(res1[k]["t2_s"]).reshape(128, VSH // 128, C)
         .transpose(1, 0, 2).reshape(VSH, C) for k in range(NCORES)], axis=0)
    t2_full = np.ascontiguousarray(t2_full, dtype=np.float32) * (1.0 / 512.0)

    # ---- launch 2: main kernel, batch-sharded ----
    t2_bf = t2_full.astype(BF)
    # token permutation: G column (th*128+p) holds token t = (col%S)*L + col//S
    cols = np.arange(T)
    tperm = (cols % S) * L + cols // S
    xp = x[:, tperm].copy()
    in2 = []
    for k in range(NCORES):
        sl = slice(k * BL, (k + 1) * BL)
        xt = xp[sl].reshape(BL, T // 128, 128).transpose(2, 1, 0) \
                   .reshape(128, T // 128 * BL)
        in2.append({
            "x_t": np.ascontiguousarray(xt),
            "t2": t2_bf,
            "blockP": prep["blockP"], "blockPT": prep["blockPT"],
            "logcolsum": prep["logcolsum"], "sadj": prep["sadj"],
        })
    res2 = _run(_CACHE["main"], in2, "main")

    # ---- host combine (float64) ----
    lengths = (x != 0).sum(1)
    start64 = start.astype(np.float64)
    end64 = end.astype(np.float64)
    fcb64 = fc_b.astype(np.float64)
    trans64 = trans.astype(np.float64)
    Pe = prep["P_eff"]
    colsum64 = Pe.sum(0)
    t264 = t2_full.astype(np.float64)
    exp_end = np.exp(end64)

    # numerator: gold-path score, fully vectorized on host
    maskf = (x != 0).astype(np.float64)
    em_tag = t264[x, tags] + fcb64[tags]           # (B,T)
    num = start64[tags[:, 0]] + (em_tag * maskf).sum(1)
    num += (trans64[tags[:, :-1], tags[:, 1:]] * maskf[:, 1:]).sum(1)
    last_tags = tags[np.arange(B), lengths - 1]
    num += end64[last_tags]

    total = 0.0
    for core in range(NCORES):
        r = np.asarray(res2[core]["r_out"], np.float64).reshape(BL, C, S)
        d = np.asarray(res2[core]["d_out"], np.float64).reshape(BL, C, S)
        d = d / colsum64[None, :, None]
        c = np.einsum('ij,bjs->bis', Pe, d)
        A = np.einsum('bis,bis->bs', r[:, :, :-1], c[:, :, 1:])   # junction s=1..S-1
        Bs = r.sum(axis=1)                                        # (BL, S)
        J = np.log(A) - np.log(Bs[:, 1:])                         # J[:, s-1] <-> junction s
        Jcum = np.concatenate([np.zeros((BL, 1)), np.cumsum(J, axis=1)], axis=1)
        for b in range(BL):
            gb = core * BL + b
            ln = int(lengths[gb])
            sstar = (ln - 1) // L
            logZ = Jcum[b, sstar - 1]        # junctions s=1..sstar-1
            alpha = r[b, :, sstar - 1].copy()
            for t in range(sstar * L, ln):
                w = np.exp(t264[x[gb, t]] + fcb64)
                alpha = (alpha @ Pe) * w
            logZ += np.log(alpha @ exp_end)
            total += -(num[gb] - logZ)
    return np.array(total, dtype=np.float32)
